# revision 10
# baseline (speedup 1.0000x reference)
"""M2 convection (SE(2) trilinear warp) Trainium2 kernel.

out[b,c,k,i,j] = x[b,c] trilinearly sampled at (theta_k, i, j) . g0[c]^{-1}.

Structure exploited: for fixed (c,k) the warp is a uniform translation —
theta taps are two whole slices (a_k, a_k+1) with constant weights, the y
taps are a per-row integer shift + 2-tap blend (exactly encoded in a banded
matrix applied on the PE, theta weight folded in), and the x taps are a
free-dim shift + 2-tap blend. Runtime-register APs are unavailable on this
execution path, so the x 2-tap blend is computed at every candidate shift
(fixed taps j, j+1 over a zero-padded PSUM tile) and the host selects each
(c,k)'s shifted window from a slightly padded output.

Weight folding: the y matrices carry wt0 (theta tap-0 weight) and
c0 = 1-fmid (x tap-0 weight), so the theta and x blends are each a single
scalar_tensor_tensor with ratio scalars ft/wt0 and fmid/c0. Matmuls run in
float32r (full-rate PE mode; ~1e-3 relative precision, far inside the 2e-2
gate).

Sharding: channels across 8 cores (2 channels/core, no communication).
The kernel's slice-slot m corresponds to output k with a_k == m; the host
unpermutes along theta at the end.
"""
import os
import sys
import numpy as np

sys.path.insert(0, "/opt/trn_rl_repo")

import concourse.mybir as mybir  # noqa: E402
from concourse import bacc, bass_utils  # noqa: E402
from concourse.tile import TileContext  # noqa: E402

TWO_PI = 2.0 * np.pi
B, C, Or, H, W = 4, 16, 8, 256, 256
N_CORES = 8
C_LOC = C // N_CORES          # channels per core
N_CK = C_LOC * Or             # (c_local, m) pairs per core


def _reference_tables(g0):
    """Replicate the reference's f32 index/weight math (jax on CPU so the
    rounding matches the jax reference bit-for-bit)."""
    import jax
    import jax.numpy as jnp

    with jax.default_device(jax.devices("cpu")[0]):
        g0 = jnp.asarray(g0, dtype=jnp.float32)
        x0, y0, th0 = g0[:, 0], g0[:, 1], g0[:, 2]
        k = jnp.arange(Or, dtype=jnp.float32)
        alpha = k[None, :] * (TWO_PI / Or) - th0[:, None]
        ca, sa = jnp.cos(alpha), jnp.sin(alpha)
        dx = ca * x0[:, None] - sa * y0[:, None]
        dy = sa * x0[:, None] + ca * y0[:, None]
        t = k[None, :] - th0[:, None] * (Or / TWO_PI)
        xs = jnp.arange(W, dtype=jnp.float32)[None, None, :] - dx[:, :, None]
        ys = jnp.arange(H, dtype=jnp.float32)[None, None, :] - dy[:, :, None]
        tf = jnp.floor(t)
        ft = t - tf
        t0i = tf.astype(jnp.int32)
        xf = jnp.floor(xs)
        fx = xs - xf
        x0i = xf.astype(jnp.int32)
        yf = jnp.floor(ys)
        fy = ys - yf
        y0i = yf.astype(jnp.int32)
        return dict(
            ft=np.asarray(ft), t0i=np.asarray(t0i),
            fx=np.asarray(fx), x0i=np.asarray(x0i),
            fy=np.asarray(fy), y0i=np.asarray(y0i),
        )


def _x_shift(tabs, c, k):
    return int(tabs["x0i"][c, k][W // 2]) - W // 2


def _pads(tabs):
    hs = [_x_shift(tabs, c, k) for c in range(C) for k in range(Or)]
    padl = -min(hs) + 2
    padr = max(hs) + 1 + 2
    return max(padl, 2), max(padr, 2)


def _core_tables(tabs, channels, padl, nu):
    """Build per-core kernel input tensors from the reference tables.

    Returns (mats, rscal, wxr, hvals, slot_to_k): slot_to_k[c_local][m] is
    the output-theta index computed in slice-slot m; hvals its x shift.
    """
    mats = np.zeros((128, C_LOC, Or, 2, 2, 128), dtype=np.float32)
    rscal = np.zeros((128, N_CK), dtype=np.float32)
    wxr = np.zeros((128, N_CK), dtype=np.float32)
    hvals = np.zeros((C_LOC, Or), dtype=np.int64)
    slot_to_k = np.zeros((C_LOC, Or), dtype=np.int64)

    for cl, c in enumerate(channels):
        a = np.mod(tabs["t0i"][c], Or)          # [Or] A-slice per out-k
        assert sorted(a.tolist()) == list(range(Or)), f"theta map not a bijection: {a}"
        k_of_m = np.zeros(Or, dtype=np.int64)
        k_of_m[a] = np.arange(Or)
        slot_to_k[cl] = k_of_m
        for m in range(Or):
            k = int(k_of_m[m])
            cki = cl * Or + m
            ft = np.float32(tabs["ft"][c, k])
            wt0 = np.float32(1.0) - ft
            # blend: t = slot_m + r * slot_{m+1}; (1-ft) folded into mats
            rscal[:, cki] = np.float32(ft / wt0) if wt0 > 0 else np.float32(0)
            # --- x scalars (c0 = 1-fmid folded into mats) ---
            x0i = tabs["x0i"][c, k]             # [W] int
            fx = tabs["fx"][c, k]               # [W] f32
            h = _x_shift(tabs, c, k)
            nonuni = np.abs(x0i - (np.arange(W) + h)).max()
            assert nonuni <= 1, f"x shift non-uniformity {nonuni} too large"
            fmid = np.float32(0.5) * (fx.min() + fx.max())
            c0 = np.float32(1.0) - fmid
            wxr[:, cki] = np.float32(fmid / c0)
            assert 0 <= padl + h and padl + h + 1 + W <= nu, f"x shift {h} vs pads"
            hvals[cl, m] = h
            # --- y matrices (per-row exact; wt0 and c0 folded in) ---
            y0i = tabs["y0i"][c, k]             # [H] int
            fy = tabs["fy"][c, k]               # [H] f32
            for dyc in (0, 1):
                wrow = (fy if dyc else (np.float32(1.0) - fy)).astype(np.float32)
                wrow = (wrow * wt0 * c0).astype(np.float32)
                r = y0i + dyc                    # src row per out row i
                valid = (r >= 0) & (r < H)
                i_idx = np.nonzero(valid)[0]
                rv = r[i_idx]
                mats[rv % 128, cl, m, i_idx // 128, rv // 128, i_idx % 128] += \
                    wrow[i_idx]
    return mats, rscal, wxr, hvals, slot_to_k


def _build_program(padl, padr):
    nu = W + padl + padr        # padded PSUM width
    nv = nu - 1                 # output candidate width
    nc = bacc.Bacc("TRN2", num_devices=N_CORES)
    f32 = mybir.dt.float32
    f32r = mybir.dt.float32r
    x_d = nc.dram_tensor("xs", [B, C_LOC, Or, H, W], f32, kind="ExternalInput")
    m_d = nc.dram_tensor("mats", [128, C_LOC, Or, 2, 2, 128], f32, kind="ExternalInput")
    r_d = nc.dram_tensor("rscal", [128, N_CK], f32, kind="ExternalInput")
    w_d = nc.dram_tensor("wx", [128, N_CK], f32, kind="ExternalInput")
    o_d = nc.dram_tensor("o", [B, C_LOC, Or, H, nv], f32, kind="ExternalOutput")

    with TileContext(nc) as tc:
        with tc.tile_pool(name="const", bufs=1) as cpool, \
             tc.tile_pool(name="xin", bufs=2) as xpool, \
             tc.tile_pool(name="work", bufs=4) as wpool, \
             tc.tile_pool(name="oout", bufs=2) as opool, \
             tc.tile_pool(name="psum", bufs=1, space="PSUM") as psum:
            mt = cpool.tile([128, C_LOC, Or, 2, 2, 128], f32r)
            rt = cpool.tile([128, N_CK], f32)
            wt = cpool.tile([128, N_CK], f32)
            # cl=0 matrices land first so the first matmuls aren't blocked
            # behind the whole 4 MB constant load.
            for cl in range(C_LOC):
                nc.sync.dma_start(out=mt[:, cl], in_=m_d.ap()[:, cl].bitcast(f32r))
            nc.sync.dma_start(out=rt[:], in_=r_d.ap())
            nc.sync.dma_start(out=wt[:], in_=w_d.ap())

            # 4 persistent PSUM tiles of 2 banks each (one bank per u half).
            # The matmuls only ever write [padl, padl+W), so the pad columns
            # are zeroed once here and stay zero across all reuses.
            U_tiles = []
            for i in range(4):
                U = psum.tile([128, 2, 512], f32, tag=f"U{i}", name=f"U{i}")
                for u in range(2):
                    nc.scalar.memzero(U[:, u, 0:padl])
                    nc.scalar.memzero(U[:, u, padl + W:nu])
                U_tiles.append(U)
            uidx = 0

            for b in range(B):
                for cl in range(C_LOC):
                    x_sb = xpool.tile([128, Or, 2, W], f32, tag="x_sb", name="x_sb")
                    src = x_d.ap()[b, cl].rearrange("k (u p) j -> p k u j", p=128)
                    nc.sync.dma_start(out=x_sb[:], in_=src)
                    out_sb = opool.tile([128, Or, 2, nv], f32, tag="out_sb",
                                        name="out_sb")
                    for m in range(Or):
                        cki = cl * Or + m
                        t = wpool.tile([128, 2, W], f32r, tag="t", name="t")
                        nc.vector.scalar_tensor_tensor(
                            out=t[:], in0=x_sb[:, (m + 1) % Or],
                            scalar=rt[:, cki:cki + 1], in1=x_sb[:, m],
                            op0=mybir.AluOpType.mult, op1=mybir.AluOpType.add)
                        U = U_tiles[uidx % 4]
                        uidx += 1
                        for u in range(2):
                            for v in range(2):
                                nc.tensor.matmul(
                                    U[:, u, padl:padl + W],
                                    mt[:, cl, m, u, v],
                                    t[:, v],
                                    start=(v == 0), stop=(v == 1))
                        # DVE reads PSUM at half rate (and may read only one
                        # PSUM operand), so ScalarE stages the whole padded
                        # row to SBUF and the DVE x-blend reads SBUF only.
                        V = wpool.tile([128, 2, nu], f32, tag="V", name="V")
                        nc.scalar.copy(V[:], U[:, :, 0:nu])
                        nc.vector.scalar_tensor_tensor(
                            out=out_sb[:, m], in0=V[:, :, 1:nu],
                            scalar=wt[:, cki:cki + 1], in1=V[:, :, 0:nv],
                            op0=mybir.AluOpType.mult, op1=mybir.AluOpType.add)
                    dst = o_d.ap()[b, cl].rearrange("k (u p) j -> p k u j", p=128)
                    nc.sync.dma_start(out=dst, in_=out_sb[:])
    nc.compile()
    return nc


_NC_CACHE = {}


def kernel(x, g0):
    x = np.ascontiguousarray(np.asarray(x, dtype=np.float32))
    g0 = np.asarray(g0, dtype=np.float32)
    tabs = _reference_tables(g0)
    padl, padr = _pads(tabs)
    nu = W + padl + padr

    if (padl, padr) not in _NC_CACHE:
        _NC_CACHE[(padl, padr)] = _build_program(padl, padr)
    nc = _NC_CACHE[(padl, padr)]

    in_maps = []
    slot_maps = []
    for core in range(N_CORES):
        channels = list(range(core * C_LOC, (core + 1) * C_LOC))
        mats, rscal, wxr, hvals, slot_to_k = _core_tables(tabs, channels, padl, nu)
        in_maps.append({
            "xs": np.ascontiguousarray(x[:, channels[0]:channels[-1] + 1]),
            "mats": mats, "rscal": rscal, "wx": wxr,
        })
        slot_maps.append((slot_to_k, hvals))

    res = bass_utils.run_bass_kernel_spmd(
        nc, in_maps, core_ids=list(range(N_CORES)),
        trace=bool(int(os.environ.get("KERNEL_TRACE", "0"))))
    kernel.last_results = res

    out = np.empty((B, C, Or, H, W), dtype=np.float32)
    for core in range(N_CORES):
        raw = res.results[core]["o"]            # [B, C_LOC, Or, H, nv], slot m
        slot_to_k, hvals = slot_maps[core]
        for cl in range(C_LOC):
            c = core * C_LOC + cl
            for m in range(Or):
                k = int(slot_to_k[cl, m])
                s = padl + int(hvals[cl, m])
                out[:, c, k] = raw[:, cl, m, :, s:s + W]
    return out


# revision 12
# speedup vs baseline: 1.3657x; 1.3657x over previous
"""M2 convection (SE(2) trilinear warp) Trainium2 kernel.

out[b,c,k,i,j] = x[b,c] trilinearly sampled at (theta_k, i, j) . g0[c]^{-1}.

Structure exploited: for fixed (c,k) the warp is a uniform translation —
theta taps are two whole slices (a_k, a_k+1) with constant weights, the y
taps are a per-row integer shift + 2-tap blend (exactly encoded in a banded
matrix applied on the PE, theta weight folded in), and the x taps are a
free-dim shift + 2-tap blend. Runtime-register APs are unavailable on this
execution path, so the x 2-tap blend is computed at every candidate shift
(fixed taps j, j+1 over a zero-padded PSUM tile) and the host selects each
(c,k)'s shifted window from a slightly padded output.

Weight folding: the y matrices carry wt0 (theta tap-0 weight) and
c0 = 1-fmid (x tap-0 weight), so the theta and x blends are each a single
scalar_tensor_tensor with ratio scalars ft/wt0 and fmid/c0. Matmuls run in
float32r (full-rate PE mode; ~1e-3 relative precision, far inside the 2e-2
gate).

Sharding: channels across 8 cores (2 channels/core, no communication).
The kernel's slice-slot m corresponds to output k with a_k == m; the host
unpermutes along theta at the end.
"""
import os
import sys
import numpy as np

sys.path.insert(0, "/opt/trn_rl_repo")

import concourse.mybir as mybir  # noqa: E402
from concourse import bacc, bass_utils  # noqa: E402
from concourse.tile import TileContext  # noqa: E402

TWO_PI = 2.0 * np.pi
B, C, Or, H, W = 4, 16, 8, 256, 256
N_CORES = 8
C_LOC = C // N_CORES          # channels per core
N_CK = C_LOC * Or             # (c_local, m) pairs per core


def _reference_tables(g0):
    """Replicate the reference's f32 index/weight math (jax on CPU so the
    rounding matches the jax reference bit-for-bit)."""
    import jax
    import jax.numpy as jnp

    with jax.default_device(jax.devices("cpu")[0]):
        g0 = jnp.asarray(g0, dtype=jnp.float32)
        x0, y0, th0 = g0[:, 0], g0[:, 1], g0[:, 2]
        k = jnp.arange(Or, dtype=jnp.float32)
        alpha = k[None, :] * (TWO_PI / Or) - th0[:, None]
        ca, sa = jnp.cos(alpha), jnp.sin(alpha)
        dx = ca * x0[:, None] - sa * y0[:, None]
        dy = sa * x0[:, None] + ca * y0[:, None]
        t = k[None, :] - th0[:, None] * (Or / TWO_PI)
        xs = jnp.arange(W, dtype=jnp.float32)[None, None, :] - dx[:, :, None]
        ys = jnp.arange(H, dtype=jnp.float32)[None, None, :] - dy[:, :, None]
        tf = jnp.floor(t)
        ft = t - tf
        t0i = tf.astype(jnp.int32)
        xf = jnp.floor(xs)
        fx = xs - xf
        x0i = xf.astype(jnp.int32)
        yf = jnp.floor(ys)
        fy = ys - yf
        y0i = yf.astype(jnp.int32)
        return dict(
            ft=np.asarray(ft), t0i=np.asarray(t0i),
            fx=np.asarray(fx), x0i=np.asarray(x0i),
            fy=np.asarray(fy), y0i=np.asarray(y0i),
        )


def _x_shift(tabs, c, k):
    return int(tabs["x0i"][c, k][W // 2]) - W // 2


def _pads(tabs):
    hs = [_x_shift(tabs, c, k) for c in range(C) for k in range(Or)]
    padl = -min(hs) + 2
    padr = max(hs) + 1 + 2
    return max(padl, 2), max(padr, 2)


def _core_tables(tabs, channels, padl, nu):
    """Build per-core kernel input tensors from the reference tables.

    Returns (mats, rscal, wxr, hvals, slot_to_k): slot_to_k[c_local][m] is
    the output-theta index computed in slice-slot m; hvals its x shift.
    """
    mats = np.zeros((128, C_LOC, Or, 2, 2, 128), dtype=np.float32)
    rscal = np.zeros((128, N_CK), dtype=np.float32)
    wxr = np.zeros((128, N_CK), dtype=np.float32)
    hvals = np.zeros((C_LOC, Or), dtype=np.int64)
    slot_to_k = np.zeros((C_LOC, Or), dtype=np.int64)

    for cl, c in enumerate(channels):
        a = np.mod(tabs["t0i"][c], Or)          # [Or] A-slice per out-k
        assert sorted(a.tolist()) == list(range(Or)), f"theta map not a bijection: {a}"
        k_of_m = np.zeros(Or, dtype=np.int64)
        k_of_m[a] = np.arange(Or)
        slot_to_k[cl] = k_of_m
        for m in range(Or):
            k = int(k_of_m[m])
            cki = cl * Or + m
            ft = np.float32(tabs["ft"][c, k])
            wt0 = np.float32(1.0) - ft
            # blend: t = slot_m + r * slot_{m+1}; (1-ft) folded into mats
            rscal[:, cki] = np.float32(ft / wt0) if wt0 > 0 else np.float32(0)
            # --- x scalars (c0 = 1-fmid folded into mats) ---
            x0i = tabs["x0i"][c, k]             # [W] int
            fx = tabs["fx"][c, k]               # [W] f32
            h = _x_shift(tabs, c, k)
            nonuni = np.abs(x0i - (np.arange(W) + h)).max()
            assert nonuni <= 1, f"x shift non-uniformity {nonuni} too large"
            fmid = np.float32(0.5) * (fx.min() + fx.max())
            c0 = np.float32(1.0) - fmid
            wxr[:, cki] = np.float32(fmid / c0)
            assert 0 <= padl + h and padl + h + 1 + W <= nu, f"x shift {h} vs pads"
            hvals[cl, m] = h
            # --- y matrices (per-row exact; wt0 and c0 folded in) ---
            y0i = tabs["y0i"][c, k]             # [H] int
            fy = tabs["fy"][c, k]               # [H] f32
            for dyc in (0, 1):
                wrow = (fy if dyc else (np.float32(1.0) - fy)).astype(np.float32)
                wrow = (wrow * wt0 * c0).astype(np.float32)
                r = y0i + dyc                    # src row per out row i
                valid = (r >= 0) & (r < H)
                i_idx = np.nonzero(valid)[0]
                rv = r[i_idx]
                mats[rv % 128, cl, m, i_idx // 128, rv // 128, i_idx % 128] += \
                    wrow[i_idx]
    return mats, rscal, wxr, hvals, slot_to_k


def _build_program(padl, padr):
    nu = W + padl + padr        # padded PSUM width
    nv = nu - 1                 # output candidate width
    nc = bacc.Bacc("TRN2", num_devices=N_CORES)
    f32 = mybir.dt.float32
    f32r = mybir.dt.float32r
    x_d = nc.dram_tensor("xs", [B, C_LOC, Or, H, W], f32, kind="ExternalInput")
    m_d = nc.dram_tensor("mats", [128, C_LOC, Or, 2, 2, 128], f32, kind="ExternalInput")
    r_d = nc.dram_tensor("rscal", [128, N_CK], f32, kind="ExternalInput")
    w_d = nc.dram_tensor("wx", [128, N_CK], f32, kind="ExternalInput")
    o_d = nc.dram_tensor("o", [B, C_LOC, Or, H, nv], f32, kind="ExternalOutput")

    with TileContext(nc) as tc:
        with tc.tile_pool(name="const", bufs=1) as cpool, \
             tc.tile_pool(name="xin", bufs=2) as xpool, \
             tc.tile_pool(name="work", bufs=4) as wpool, \
             tc.tile_pool(name="oout", bufs=2) as opool, \
             tc.tile_pool(name="psum", bufs=1, space="PSUM") as psum:
            mt = cpool.tile([128, C_LOC, Or, 2, 2, 128], f32r)
            rt = cpool.tile([128, N_CK], f32)
            wt = cpool.tile([128, N_CK], f32)
            # cl=0 matrices land first so the first matmuls aren't blocked
            # behind the whole 4 MB constant load.
            for cl in range(C_LOC):
                nc.sync.dma_start(out=mt[:, cl], in_=m_d.ap()[:, cl].bitcast(f32r))
            nc.sync.dma_start(out=rt[:], in_=r_d.ap())
            nc.sync.dma_start(out=wt[:], in_=w_d.ap())

            # 4 persistent PSUM tiles of 2 banks each (one bank per u half).
            # The matmuls only ever write [padl, padl+W), so the pad columns
            # are zeroed once here and stay zero across all reuses.
            U_tiles = []
            for i in range(4):
                U = psum.tile([128, 2, 512], f32, tag=f"U{i}", name=f"U{i}")
                for u in range(2):
                    nc.scalar.memzero(U[:, u, 0:padl])
                    nc.scalar.memzero(U[:, u, padl + W:nu])
                U_tiles.append(U)
            uidx = 0

            for b in range(B):
                for cl in range(C_LOC):
                    x_sb = xpool.tile([128, Or, 2, W], f32, tag="x_sb", name="x_sb")
                    src = x_d.ap()[b, cl].rearrange("k (u p) j -> p k u j", p=128)
                    nc.sync.dma_start(out=x_sb[:], in_=src)
                    out_sb = opool.tile([128, Or, 2, nv], f32, tag="out_sb",
                                        name="out_sb")
                    # All theta blends first: keeps the vector queue from
                    # stalling behind x-blends that wait on matmul+copy.
                    t_tiles = []
                    for m in range(Or):
                        cki = cl * Or + m
                        t = wpool.tile([128, 2, W], f32r, tag="t", name="t")
                        nc.vector.scalar_tensor_tensor(
                            out=t[:], in0=x_sb[:, (m + 1) % Or],
                            scalar=rt[:, cki:cki + 1], in1=x_sb[:, m],
                            op0=mybir.AluOpType.mult, op1=mybir.AluOpType.add)
                        t_tiles.append(t)
                    for m in range(Or):
                        cki = cl * Or + m
                        t = t_tiles[m]
                        U = U_tiles[uidx % 4]
                        uidx += 1
                        for u in range(2):
                            for v in range(2):
                                nc.tensor.matmul(
                                    U[:, u, padl:padl + W],
                                    mt[:, cl, m, u, v],
                                    t[:, v],
                                    start=(v == 0), stop=(v == 1))
                        # DVE reads PSUM at half rate (and may read only one
                        # PSUM operand), so ScalarE stages the whole padded
                        # row to SBUF and the DVE x-blend reads SBUF only.
                        V = wpool.tile([128, 2, nu], f32, tag="V", name="V")
                        nc.scalar.copy(V[:], U[:, :, 0:nu])
                        nc.vector.scalar_tensor_tensor(
                            out=out_sb[:, m], in0=V[:, :, 1:nu],
                            scalar=wt[:, cki:cki + 1], in1=V[:, :, 0:nv],
                            op0=mybir.AluOpType.mult, op1=mybir.AluOpType.add)
                    dst = o_d.ap()[b, cl].rearrange("k (u p) j -> p k u j", p=128)
                    nc.sync.dma_start(out=dst, in_=out_sb[:])
    nc.compile()
    return nc


_NC_CACHE = {}


def kernel(x, g0):
    x = np.ascontiguousarray(np.asarray(x, dtype=np.float32))
    g0 = np.asarray(g0, dtype=np.float32)
    tabs = _reference_tables(g0)
    padl, padr = _pads(tabs)
    nu = W + padl + padr

    if (padl, padr) not in _NC_CACHE:
        _NC_CACHE[(padl, padr)] = _build_program(padl, padr)
    nc = _NC_CACHE[(padl, padr)]

    in_maps = []
    slot_maps = []
    for core in range(N_CORES):
        channels = list(range(core * C_LOC, (core + 1) * C_LOC))
        mats, rscal, wxr, hvals, slot_to_k = _core_tables(tabs, channels, padl, nu)
        in_maps.append({
            "xs": np.ascontiguousarray(x[:, channels[0]:channels[-1] + 1]),
            "mats": mats, "rscal": rscal, "wx": wxr,
        })
        slot_maps.append((slot_to_k, hvals))

    res = bass_utils.run_bass_kernel_spmd(
        nc, in_maps, core_ids=list(range(N_CORES)),
        trace=bool(int(os.environ.get("KERNEL_TRACE", "0"))))
    kernel.last_results = res

    out = np.empty((B, C, Or, H, W), dtype=np.float32)
    for core in range(N_CORES):
        raw = res.results[core]["o"]            # [B, C_LOC, Or, H, nv], slot m
        slot_to_k, hvals = slot_maps[core]
        for cl in range(C_LOC):
            c = core * C_LOC + cl
            for m in range(Or):
                k = int(slot_to_k[cl, m])
                s = padl + int(hvals[cl, m])
                out[:, c, k] = raw[:, cl, m, :, s:s + W]
    return out
